# revision 1
# baseline (speedup 1.0000x reference)
"""Trainium2 Bass kernel for nn_AttentionHead (B=16, T=2048, DIM=512, HEAD=64).

Strategy: data-parallel over batch across 8 NeuronCores (2 batches/core).
Host-side prep (free): x is pre-transposed to [DIM, T] bf16 per batch, the
Wq/Wk projection weights are stacked so one matmul produces [Q^T; K^T], and
the rotary coefficient tables are expanded to [128, T] with the pair-swap
handled by a permutation matmul on-device.

Per-core graph (per batch):
  QK^T = Wqk^T @ x^T (PE, bf16)            -> [128, T] psum
  swap = P_pairswap @ QK^T (PE, f32r)
  q~/k~ = QK^T*fr + swap*fi (DVE, f32)     -> rotary applied, [64, T] each
  S^T[j-chunk] = k~[j].T @ q~ (PE, f32r)   -> [128, 1024] psum per chunk
  P^T = exp(S^T / sqrt(512)) (ACT, fp16)   -> no max-subtraction needed:
        |S|/sqrt(512) <= ~5 so exp is safely bounded in fp16/f32
  out^T += V~[j].T @ P^T (PE, fp16)        -> V~ has a ones column so row 64
        accumulates the softmax denominator
  transpose out^T chunks (PE) + multiply by 1/rowsum (DVE) -> out [T, 64] f32
"""

import os
import sys

for _p in ("/opt/trn_rl_repo", "/root/.axon_site/_ro/trn_rl_repo"):
    if os.path.isdir(_p) and _p not in sys.path:
        sys.path.append(_p)

import numpy as np
import ml_dtypes

import concourse.bass as bass
import concourse.mybir as mybir
import concourse.tile as tile
from concourse import bacc
from concourse.bass import ts
from concourse.bass_utils import run_bass_kernel_spmd

F32 = mybir.dt.float32
F32R = mybir.dt.float32r
BF16 = mybir.dt.bfloat16
F16 = mybir.dt.float16

B, T, DIM, HEAD = 16, 2048, 512, 64
NCORES = 8
BPC = B // NCORES          # batches per core
NCC = DIM // 128           # contraction chunks
NT = T // 512              # 512-wide tiles along t
NJ = T // 128              # key chunks
IH = T // 1024             # query halves
IW = 1024                  # query half width
NIB = IW // 512


def _build():
    scale = 1.0 / np.sqrt(np.float32(DIM))
    nc = bacc.Bacc(None, target_bir_lowering=False)
    xt_e = nc.declare_dram_parameter("xt", [BPC, DIM, T], BF16, isOutput=False)
    wqk_e = nc.declare_dram_parameter("wqk", [NCC, 128, 128], BF16, isOutput=False)
    wv_e = nc.declare_dram_parameter("wv", [NCC, 128, HEAD], BF16, isOutput=False)
    perm_e = nc.declare_dram_parameter("perm", [128, 128], F32R, isOutput=False)
    fr_e = nc.declare_dram_parameter("fr", [128, T], F32, isOutput=False)
    fi_e = nc.declare_dram_parameter("fi", [128, T], F32, isOutput=False)
    id_e = nc.declare_dram_parameter("ident", [128, 128], F32, isOutput=False)
    out_e = nc.declare_dram_parameter("out", [BPC, T, HEAD], F32, isOutput=True)

    with tile.TileContext(nc) as tc:
        with (
            tc.tile_pool(name="consts", bufs=1) as cp,
            tc.tile_pool(name="xt", bufs=2) as xp,
            tc.tile_pool(name="big", bufs=2) as bp,
            tc.tile_pool(name="vtp", bufs=2) as vp,
            tc.tile_pool(name="ptp", bufs=3) as pp,
            tc.tile_pool(name="op", bufs=2) as op,
            tc.tile_pool(name="oo", bufs=4) as oop,
            tc.tile_pool(name="psA", bufs=2, space="PSUM") as psA,
            tc.tile_pool(name="psS", bufs=2, space="PSUM") as psS,
            tc.tile_pool(name="psO", bufs=1, space="PSUM") as psO,
        ):
            wqk_t, wv_t = [], []
            for ci in range(NCC):
                w1 = cp.tile([128, 128], BF16, tag=f"wqk{ci}")
                nc.sync.dma_start(out=w1, in_=wqk_e[ci])
                wqk_t.append(w1)
                w2 = cp.tile([128, HEAD], BF16, tag=f"wv{ci}")
                nc.sync.dma_start(out=w2, in_=wv_e[ci])
                wv_t.append(w2)
            perm_t = cp.tile([128, 128], F32R, tag="perm")
            nc.sync.dma_start(out=perm_t, in_=perm_e[:])
            fr_t = cp.tile([128, T], F32, tag="fr")
            nc.sync.dma_start(out=fr_t, in_=fr_e[:])
            fi_t = cp.tile([128, T], F32, tag="fi")
            nc.sync.dma_start(out=fi_t, in_=fi_e[:])
            id32 = cp.tile([128, 128], F32, tag="id32")
            nc.sync.dma_start(out=id32, in_=id_e[:])
            id16 = cp.tile([128, 128], F16, tag="id16")
            nc.vector.tensor_copy(id16, id32)

            for b in range(BPC):
                xts = []
                for ci in range(NCC):
                    xt = xp.tile([128, T], BF16, tag=f"xt{ci}")
                    nc.sync.dma_start(out=xt, in_=xt_e[b, ts(ci, 128)])
                    xts.append(xt)

                qk_s = bp.tile([128, T], F32R, tag="qk")
                vT_s = bp.tile([HEAD, T], F16, tag="vT")
                for tt in range(NT):
                    pqk = psA.tile([128, 512], F32, tag="a")
                    for ci in range(NCC):
                        nc.tensor.matmul(pqk, wqk_t[ci], xts[ci][:, ts(tt, 512)],
                                         start=(ci == 0), stop=(ci == NCC - 1))
                    nc.scalar.copy(out=qk_s[:, ts(tt, 512)], in_=pqk)
                    pv = psA.tile([HEAD, 512], F32, tag="a")
                    for ci in range(NCC):
                        nc.tensor.matmul(pv, wv_t[ci], xts[ci][:, ts(tt, 512)],
                                         start=(ci == 0), stop=(ci == NCC - 1))
                    nc.vector.tensor_copy(vT_s[:, ts(tt, 512)], pv)

                t1 = bp.tile([128, T], F32, tag="t1")
                nc.vector.tensor_tensor(out=t1, in0=qk_s.bitcast(F32), in1=fr_t,
                                        op=mybir.AluOpType.mult)
                t2 = bp.tile([128, T], F32, tag="t2")
                for tt in range(NT):
                    psw = psA.tile([128, 512], F32, tag="a")
                    nc.tensor.matmul(psw, perm_t, qk_s[:, ts(tt, 512)],
                                     start=True, stop=True)
                    nc.vector.tensor_tensor(out=t2[:, ts(tt, 512)], in0=psw,
                                            in1=fi_t[:, ts(tt, 512)],
                                            op=mybir.AluOpType.mult)
                qr = bp.tile([64, T], F32R, tag="qr")
                nc.vector.tensor_tensor(out=qr, in0=t1[0:64, :], in1=t2[0:64, :],
                                        op=mybir.AluOpType.add)
                kr = bp.tile([64, T], F32R, tag="kr")
                nc.vector.tensor_tensor(out=kr, in0=t1[64:128, :],
                                        in1=t2[64:128, :],
                                        op=mybir.AluOpType.add)

                vts = []
                for j in range(NJ):
                    pvt = psA.tile([128, HEAD], F16, tag="a")
                    nc.tensor.transpose(pvt, vT_s[:, ts(j, 128)],
                                        id16[0:HEAD, 0:HEAD])
                    vt = vp.tile([128, HEAD + 1], F16, tag=f"vt{j}")
                    nc.vector.tensor_copy(vt[:, 0:HEAD], pvt)
                    nc.vector.memset(vt[:, HEAD:HEAD + 1], 1.0)
                    vts.append(vt)

                for ih in range(IH):
                    po = psO.tile([HEAD + 1, IW], F32, tag="po")
                    for j in range(NJ):
                        ps = psS.tile([128, IW], F32, tag="s")
                        for ib in range(NIB):
                            nc.tensor.matmul(ps[:, ts(ib, 512)],
                                             kr[:, ts(j, 128)],
                                             qr[:, ts(ih * NIB + ib, 512)],
                                             start=True, stop=True)
                        pT = pp.tile([128, IW], F16, tag="pT")
                        nc.scalar.activation(
                            out=pT, in_=ps,
                            func=mybir.ActivationFunctionType.Exp,
                            scale=float(scale))
                        for ib in range(NIB):
                            nc.tensor.matmul(po[:, ts(ib, 512)], vts[j],
                                             pT[:, ts(ib, 512)],
                                             start=(j == 0), stop=(j == NJ - 1),
                                             skip_group_check=True)
                    oc = op.tile([HEAD + 1, IW], F32, tag="oc")
                    nc.vector.tensor_copy(oc, po)
                    for tsub in range(IW // 128):
                        ptr = psA.tile([128, HEAD + 1], F32, tag="a")
                        nc.tensor.transpose(ptr, oc[:, ts(tsub, 128)],
                                            id32[0:HEAD + 1, 0:HEAD + 1])
                        rc = oop.tile([128, 1], F32, tag="rc")
                        nc.vector.reciprocal(rc, ptr[:, HEAD:HEAD + 1])
                        ot = oop.tile([128, HEAD], F32, tag="ot")
                        nc.vector.tensor_scalar_mul(out=ot, in0=ptr[:, 0:HEAD],
                                                    scalar1=rc)
                        row0 = ih * IW + tsub * 128
                        nc.sync.dma_start(out=out_e[b, row0:row0 + 128, :],
                                          in_=ot)
    nc.compile()
    return nc


def _prep_consts(Wq, Wk, Wv, fx_real, fx_imag, fy_real, fy_imag):
    WqT = np.asarray(Wq, np.float32).T
    WkT = np.asarray(Wk, np.float32).T
    WvT = np.asarray(Wv, np.float32).T
    wqk = np.concatenate([WqT, WkT], axis=1).reshape(NCC, 128, 128)
    wqk = np.ascontiguousarray(wqk).astype(ml_dtypes.bfloat16)
    wv = np.ascontiguousarray(WvT.reshape(NCC, 128, HEAD)).astype(
        ml_dtypes.bfloat16)

    perm = np.zeros((128, 128), np.float32)
    for m in range(128):
        sw = m + 1 if (m % 2 == 0) else m - 1
        perm[sw, m] = 1.0

    fx_real = np.asarray(fx_real, np.float32)
    fx_imag = np.asarray(fx_imag, np.float32)
    fy_real = np.asarray(fy_real, np.float32)
    fy_imag = np.asarray(fy_imag, np.float32)
    fr64 = np.zeros((64, T), np.float32)
    fi64 = np.zeros((64, T), np.float32)
    for h in range(64):
        if h < 32:
            frs, fis, p = fx_real, fx_imag, h // 2
        else:
            frs, fis, p = fy_real, fy_imag, (h - 32) // 2
        fr64[h] = frs[:, p]
        fi64[h] = fis[:, p] * (-1.0 if h % 2 == 0 else 1.0)
    fr = np.concatenate([fr64, fr64], axis=0)
    fi = np.concatenate([fi64, fi64], axis=0)
    ident = np.eye(128, dtype=np.float32)
    return dict(wqk=wqk, wv=wv, perm=perm, fr=fr, fi=fi, ident=ident)


_NC_CACHE = {}


def _get_nc():
    if "nc" not in _NC_CACHE:
        _NC_CACHE["nc"] = _build()
    return _NC_CACHE["nc"]


def kernel(x, Wq, Wk, Wv, fx_real, fx_imag, fy_real, fy_imag):
    x = np.asarray(x, np.float32)
    xt = np.ascontiguousarray(x.transpose(0, 2, 1)).astype(ml_dtypes.bfloat16)
    consts = _prep_consts(Wq, Wk, Wv, fx_real, fx_imag, fy_real, fy_imag)
    in_maps = []
    for c in range(NCORES):
        m = {"xt": xt[c * BPC:(c + 1) * BPC]}
        m.update(consts)
        in_maps.append(m)
    nc = _get_nc()
    res = run_bass_kernel_spmd(nc, in_maps, core_ids=list(range(NCORES)))
    out = np.concatenate([res.results[c]["out"] for c in range(NCORES)], axis=0)
    return out.astype(np.float32)


# revision 7
# speedup vs baseline: 965.8683x; 965.8683x over previous
"""Trainium2 Bass kernel for nn_AttentionHead (B=16, T=2048, DIM=512, HEAD=64).

Strategy: data-parallel over batch across 8 NeuronCores (2 batches/core).
Host-side prep (free): x is pre-transposed to [DIM, T] bf16 per batch, the
Wq/Wk projection weights are stacked so one matmul produces [Q^T; K^T], and
the rotary coefficient tables are expanded to [128, T] with the pair-swap
handled by a permutation matmul on-device.

Per-core graph (per batch):
  QK^T = Wqk^T @ x^T (PE, bf16)            -> [128, T] psum
  swap = P_pairswap @ QK^T (PE, f32r)
  q~/k~ = QK^T*fr + swap*fi (DVE, f32)     -> rotary applied, [64, T] each
  S^T[j-chunk] = k~[j].T @ q~ (PE, f32r)   -> [128, 1024] psum per chunk
  P^T = exp(S^T / sqrt(512)) (ACT, fp16)   -> no max-subtraction needed:
        |S|/sqrt(512) <= ~5 so exp is safely bounded in fp16/f32
  out^T += V~[j].T @ P^T (PE, fp16)        -> V~ has a ones column so row 64
        accumulates the softmax denominator
  transpose out^T chunks (PE) + multiply by 1/rowsum (DVE) -> out [T, 64] f32
"""

import os
import sys

for _p in ("/opt/trn_rl_repo", "/root/.axon_site/_ro/trn_rl_repo"):
    if os.path.isdir(_p) and _p not in sys.path:
        sys.path.append(_p)

import numpy as np
import ml_dtypes

import concourse.bass as bass
import concourse.mybir as mybir
import concourse.tile as tile
from concourse import bacc
from concourse.bass import ts
from concourse.bass_utils import run_bass_kernel_spmd

F32 = mybir.dt.float32
F32R = mybir.dt.float32r
BF16 = mybir.dt.bfloat16
F16 = mybir.dt.float16

B, T, DIM, HEAD = 16, 2048, 512, 64
NCORES = 8
BPC = B // NCORES          # batches per core
NCC = DIM // 128           # contraction chunks
NT = T // 512              # 512-wide tiles along t
NJ = T // 128              # key chunks
IH = T // 1024             # query halves
IW = 1024                  # query half width
NIB = IW // 512


def _build():
    scale = 1.0 / np.sqrt(np.float32(DIM))
    nc = bacc.Bacc(None, target_bir_lowering=False)
    xt_e = nc.declare_dram_parameter("xt", [BPC, DIM, T], BF16, isOutput=False)
    wqk_e = nc.declare_dram_parameter("wqk", [NCC, 128, 128], BF16, isOutput=False)
    wv_e = nc.declare_dram_parameter("wv", [NCC, 128, HEAD], BF16, isOutput=False)
    perm_e = nc.declare_dram_parameter("perm", [128, 128], F32R, isOutput=False)
    fr_e = nc.declare_dram_parameter("fr", [128, T], F32, isOutput=False)
    fi_e = nc.declare_dram_parameter("fi", [128, T], F32, isOutput=False)
    id_e = nc.declare_dram_parameter("ident", [128, 128], F32, isOutput=False)
    out_e = nc.declare_dram_parameter("out", [BPC, T, HEAD], F32, isOutput=True)

    with tile.TileContext(nc) as tc:
        with (
            tc.tile_pool(name="consts", bufs=1) as cp,
            tc.tile_pool(name="xt", bufs=2) as xp,
            tc.tile_pool(name="big", bufs=2) as bp,
            tc.tile_pool(name="vtp", bufs=2) as vp,
            tc.tile_pool(name="ptp", bufs=3) as pp,
            tc.tile_pool(name="op", bufs=2) as op,
            tc.tile_pool(name="oo", bufs=4) as oop,
            tc.tile_pool(name="psA", bufs=2, space="PSUM") as psA,
            tc.tile_pool(name="psS", bufs=2, space="PSUM") as psS,
            tc.tile_pool(name="psO", bufs=1, space="PSUM") as psO,
        ):
            wqk_t, wv_t = [], []
            for ci in range(NCC):
                w1 = cp.tile([128, 128], BF16, tag=f"wqk{ci}")
                nc.sync.dma_start(out=w1, in_=wqk_e[ci])
                wqk_t.append(w1)
                w2 = cp.tile([128, HEAD], BF16, tag=f"wv{ci}")
                nc.sync.dma_start(out=w2, in_=wv_e[ci])
                wv_t.append(w2)
            perm_t = cp.tile([128, 128], F32R, tag="perm")
            nc.sync.dma_start(out=perm_t, in_=perm_e[:])
            fr_t = cp.tile([128, T], F32, tag="fr")
            nc.sync.dma_start(out=fr_t, in_=fr_e[:])
            fi_t = cp.tile([128, T], F32, tag="fi")
            nc.sync.dma_start(out=fi_t, in_=fi_e[:])
            id32 = cp.tile([128, 128], F32, tag="id32")
            nc.sync.dma_start(out=id32, in_=id_e[:])
            id16 = cp.tile([128, 128], F16, tag="id16")
            nc.vector.tensor_copy(id16, id32)

            for b in range(BPC):
                xts = []
                for ci in range(NCC):
                    xt = xp.tile([128, T], BF16, tag=f"xt{ci}")
                    nc.sync.dma_start(out=xt, in_=xt_e[b, ts(ci, 128)])
                    xts.append(xt)

                qk_s = bp.tile([128, T], F32R, tag="qk")
                vT_s = bp.tile([HEAD, T], F16, tag="vT")
                for tt in range(NT):
                    pqk = psA.tile([128, 512], F32, tag="a")
                    for ci in range(NCC):
                        nc.tensor.matmul(pqk, wqk_t[ci], xts[ci][:, ts(tt, 512)],
                                         start=(ci == 0), stop=(ci == NCC - 1))
                    nc.vector.tensor_copy(qk_s[:, ts(tt, 512)], pqk)
                    pv = psA.tile([HEAD, 512], F32, tag="a")
                    for ci in range(NCC):
                        nc.tensor.matmul(pv, wv_t[ci], xts[ci][:, ts(tt, 512)],
                                         start=(ci == 0), stop=(ci == NCC - 1))
                    nc.vector.tensor_copy(vT_s[:, ts(tt, 512)], pv)

                t1 = bp.tile([128, T], F32, tag="t1")
                nc.vector.tensor_tensor(out=t1, in0=qk_s.bitcast(F32), in1=fr_t,
                                        op=mybir.AluOpType.mult)
                t2 = bp.tile([128, T], F32, tag="t2")
                for tt in range(NT):
                    psw = psA.tile([128, 512], F32, tag="a")
                    nc.tensor.matmul(psw, perm_t, qk_s[:, ts(tt, 512)],
                                     start=True, stop=True)
                    nc.vector.tensor_tensor(out=t2[:, ts(tt, 512)], in0=psw,
                                            in1=fi_t[:, ts(tt, 512)],
                                            op=mybir.AluOpType.mult)
                # q~ duplicated to partitions 0-63 and 64-127 (and k~ likewise)
                # so two K=64 score matmuls can run concurrently in the PE
                # array (row groups 0-1 and 2-3).
                qd = bp.tile([128, T], F32R, tag="qd")
                nc.vector.tensor_tensor(out=qd[0:64, :], in0=t1[0:64, :],
                                        in1=t2[0:64, :], op=mybir.AluOpType.add)
                nc.sync.dma_start(out=qd[64:128, :], in_=qd[0:64, :])
                kd = bp.tile([128, T], F32R, tag="kd")
                nc.vector.tensor_tensor(out=kd[64:128, :], in0=t1[64:128, :],
                                        in1=t2[64:128, :], op=mybir.AluOpType.add)
                nc.sync.dma_start(out=kd[0:64, :], in_=kd[64:128, :])

                vts = []
                for j in range(NJ):
                    pvt = psA.tile([128, HEAD], F16, tag="a")
                    nc.tensor.transpose(pvt, vT_s[:, ts(j, 128)],
                                        id16[0:HEAD, 0:HEAD])
                    vt = vp.tile([128, HEAD + 1], F16, tag=f"vt{j}")
                    nc.vector.tensor_copy(vt[:, 0:HEAD], pvt)
                    nc.vector.memset(vt[:, HEAD:HEAD + 1], 1.0)
                    vts.append(vt)

                # Attention, i-half outer / key-pair inner. Per key pair the
                # two K=64 score matmuls run row-tiled (array rows 0-63 and
                # 64-127) concurrently; one exp covers the [128, 2048] pair
                # tile ([A0|B0|A1|B1] blocks of 512 query columns).
                NP = NJ // 2
                for ih in range(IH):
                    po = psO.tile([HEAD + 1, IW], F32, tag="po")
                    for jp in range(NP):
                        jA, jB = 2 * jp, 2 * jp + 1
                        for ib in range(NIB):
                            iq = ih * NIB + ib
                            sp = psS.tile([128, 1024], F32, tag="s")
                            nc.tensor.matmul(sp[:, 0:512],
                                             kd[0:64, ts(jA, 128)],
                                             qd[0:64, ts(iq, 512)],
                                             start=True, stop=True)
                            nc.tensor.matmul(sp[:, 512:1024],
                                             kd[64:128, ts(jB, 128)],
                                             qd[64:128, ts(iq, 512)],
                                             start=True, stop=True)
                            pT = pp.tile([128, 1024], F16, tag="pT")
                            nc.scalar.activation(
                                out=pT, in_=sp,
                                func=mybir.ActivationFunctionType.Exp,
                                scale=float(scale))
                            nc.tensor.matmul(po[:, ts(ib, 512)], vts[jA],
                                             pT[:, 0:512],
                                             start=(jp == 0), stop=False,
                                             skip_group_check=True)
                            nc.tensor.matmul(po[:, ts(ib, 512)], vts[jB],
                                             pT[:, 512:1024],
                                             start=False, stop=(jp == NP - 1),
                                             skip_group_check=True)
                    oc = op.tile([HEAD + 1, IW], F32, tag="oc")
                    nc.vector.tensor_copy(oc, po)
                    for tsub in range(IW // 128):
                        ptr = psA.tile([128, HEAD + 1], F32, tag="a")
                        nc.tensor.transpose(ptr, oc[:, ts(tsub, 128)],
                                            id32[0:HEAD + 1, 0:HEAD + 1])
                        rc = oop.tile([128, 1], F32, tag="rc")
                        nc.vector.reciprocal(rc, ptr[:, HEAD:HEAD + 1])
                        ot = oop.tile([128, HEAD], F32, tag="ot")
                        nc.vector.tensor_scalar_mul(out=ot, in0=ptr[:, 0:HEAD],
                                                    scalar1=rc)
                        row0 = ih * IW + tsub * 128
                        nc.sync.dma_start(out=out_e[b, row0:row0 + 128, :],
                                          in_=ot)
    nc.compile()
    return nc


def _prep_consts(Wq, Wk, Wv, fx_real, fx_imag, fy_real, fy_imag):
    WqT = np.asarray(Wq, np.float32).T
    WkT = np.asarray(Wk, np.float32).T
    WvT = np.asarray(Wv, np.float32).T
    wqk = np.concatenate([WqT, WkT], axis=1).reshape(NCC, 128, 128)
    wqk = np.ascontiguousarray(wqk).astype(ml_dtypes.bfloat16)
    wv = np.ascontiguousarray(WvT.reshape(NCC, 128, HEAD)).astype(
        ml_dtypes.bfloat16)

    perm = np.zeros((128, 128), np.float32)
    for m in range(128):
        sw = m + 1 if (m % 2 == 0) else m - 1
        perm[sw, m] = 1.0

    fx_real = np.asarray(fx_real, np.float32)
    fx_imag = np.asarray(fx_imag, np.float32)
    fy_real = np.asarray(fy_real, np.float32)
    fy_imag = np.asarray(fy_imag, np.float32)
    fr64 = np.zeros((64, T), np.float32)
    fi64 = np.zeros((64, T), np.float32)
    for h in range(64):
        if h < 32:
            frs, fis, p = fx_real, fx_imag, h // 2
        else:
            frs, fis, p = fy_real, fy_imag, (h - 32) // 2
        fr64[h] = frs[:, p]
        fi64[h] = fis[:, p] * (-1.0 if h % 2 == 0 else 1.0)
    fr = np.concatenate([fr64, fr64], axis=0)
    fi = np.concatenate([fi64, fi64], axis=0)
    ident = np.eye(128, dtype=np.float32)
    return dict(wqk=wqk, wv=wv, perm=perm, fr=fr, fi=fi, ident=ident)


_NC_CACHE = {}


def _get_nc():
    if "nc" not in _NC_CACHE:
        _NC_CACHE["nc"] = _build()
    return _NC_CACHE["nc"]


def kernel(x, Wq, Wk, Wv, fx_real, fx_imag, fy_real, fy_imag):
    x = np.asarray(x, np.float32)
    xt = np.ascontiguousarray(x.transpose(0, 2, 1)).astype(ml_dtypes.bfloat16)
    consts = _prep_consts(Wq, Wk, Wv, fx_real, fx_imag, fy_real, fy_imag)
    in_maps = []
    for c in range(NCORES):
        m = {"xt": xt[c * BPC:(c + 1) * BPC]}
        m.update(consts)
        in_maps.append(m)
    nc = _get_nc()
    res = run_bass_kernel_spmd(nc, in_maps, core_ids=list(range(NCORES)))
    out = np.concatenate([res.results[c]["out"] for c in range(NCORES)], axis=0)
    return out.astype(np.float32)
